# revision 1
# baseline (speedup 1.0000x reference)
"""Mask2Former loss on 8 Trainium2 NeuronCores (Bass/Tile).

Strategy
--------
All heavy compute is three batched reductions over the HW=65536 mask axis,
done as PE matmuls against the (binarized-equal-raw) target masks T plus a
ones column:

    X[n,q] = sum_h pm[q,h] * T[n,h]          (raw mask logits)
    N[n,q] = sum_h nsp[q,h] * T[n,h]         nsp = ln(1 + e^{-pm}) = softplus(-pm)
    S[n,q] = sum_h sigma[q,h] * T[n,h]       sigma = sigmoid(pm) = exp(-nsp)

with the ones column (row 10) giving the plain row sums over h.  Everything
the loss needs is a function of these (identities used):

    softplus(x) = x + softplus(-x)  =>  Ps[q] = sum_h softplus(pm) = Xs[q] + Ns[q]
    BCE_sum(q,n) = sum_h [softplus(pm) - T*pm] = Ps[q] - X[n,q]
    cost_mask = BCE_sum / HW ; bce uses the matched entries of the same matrix
    dice uses S, its ones-row, and host-side target-mask sums.

Sharding: data-parallel over B (4 samples) x 2-way split of HW -> 8 cores.
Host does: input relayout (h-major tiles so no device transposes are needed),
the tiny Hungarian assignments between device stats and final scalars, and
the O(B*Q*N) loss assembly.

The activation chain uses only Exp and Ln, which share one ACT table set
(natural_log_exp_and_others), so there is no table thrashing.  Matmuls run
in float32r (full-rate PE); all matmul operands are produced either by DMA
(raw, exact fp32 bits) or by ACT writing float32r (hardware-rounded), which
satisfies the walrus fp32r verifier.
"""

import sys
import types

import numpy as np

_TRN_REPO = "/opt/trn_rl_repo"

# Problem constants (hardcoded per contract; kernel.py must be self-contained).
B, Q, NCLS, NT, H, W = 4, 100, 3, 10, 256, 256
HW = H * W
NCORES = 8
P = 128                      # SBUF partitions / h-chunk size
CHUNKS = HW // (2 * P)       # 256 h-chunks of 128 per core (half of HW)
GROUP = 32                   # chunks per DMA/ACT group
NGROUPS = CHUNKS // GROUP    # 8
NT1 = NT + 1                 # targets + ones column

_cached = {}


def _ensure_path():
    if _TRN_REPO not in sys.path:
        sys.path.insert(0, _TRN_REPO)


def _build_module():
    """Build + compile the per-core Bass module (identical on all cores)."""
    _ensure_path()
    import concourse.bacc as bacc
    import concourse.tile as tile
    from concourse import mybir

    f32 = mybir.dt.float32
    f32r = mybir.dt.float32r
    AF = mybir.ActivationFunctionType

    nc = bacc.Bacc("TRN2", target_bir_lowering=False, debug=False,
                   num_devices=NCORES)
    pmt = nc.dram_tensor("pmt", [P, CHUNKS, Q], f32r, kind="ExternalInput")
    taug = nc.dram_tensor("taug", [P, CHUNKS, NT1], f32r, kind="ExternalInput")
    stats = nc.dram_tensor("stats", [NT1, 3 * Q], f32, kind="ExternalOutput")

    with tile.TileContext(nc) as tc:
        with (
            tc.tile_pool(name="tpool", bufs=1) as tpool,
            tc.tile_pool(name="raws", bufs=3) as raws,
            tc.tile_pool(name="acts", bufs=2) as acts,
            tc.tile_pool(name="psum", bufs=1, space="PSUM") as psum,
            tc.tile_pool(name="outp", bufs=1) as outp,
        ):
            # Target masks (+ ones col) stay resident for the whole kernel.
            tt = tpool.tile([P, CHUNKS, NT1], f32r)
            nc.sync.dma_start(out=tt[:], in_=taug[:])

            ps_x = psum.tile([P, Q], f32)
            ps_n = psum.tile([P, Q], f32)
            ps_s = psum.tile([P, Q], f32)

            for g in range(NGROUPS):
                raw = raws.tile([P, GROUP, Q], f32r)
                nc.sync.dma_start(out=raw[:],
                                  in_=pmt[:, g * GROUP:(g + 1) * GROUP, :])
                e2 = acts.tile([P, GROUP, Q], f32)
                nsp = acts.tile([P, GROUP, Q], f32r)
                sig = acts.tile([P, GROUP, Q], f32r)
                # e2 = exp(-x); nsp = ln(1 + e2); sigma = exp(-nsp)
                nc.scalar.activation(out=e2[:], in_=raw[:], func=AF.Exp,
                                     scale=-1.0)
                nc.scalar.activation(out=nsp[:], in_=e2[:], func=AF.Ln,
                                     bias=1.0)
                nc.scalar.activation(out=sig[:], in_=nsp[:], func=AF.Exp,
                                     scale=-1.0)
                for c in range(GROUP):
                    k = g * GROUP + c
                    first, last = (k == 0), (k == CHUNKS - 1)
                    lhsT = tt[:, k, :]
                    nc.tensor.matmul(ps_x[0:NT1, :], lhsT, raw[:, c, :],
                                     start=first, stop=last)
                    nc.tensor.matmul(ps_n[0:NT1, :], lhsT, nsp[:, c, :],
                                     start=first, stop=last)
                    nc.tensor.matmul(ps_s[0:NT1, :], lhsT, sig[:, c, :],
                                     start=first, stop=last)

            ob = outp.tile([NT1, 3 * Q], f32)
            nc.vector.tensor_copy(ob[:, 0:Q], ps_x[0:NT1, :])
            nc.vector.tensor_copy(ob[:, Q:2 * Q], ps_n[0:NT1, :])
            nc.vector.tensor_copy(ob[:, 2 * Q:3 * Q], ps_s[0:NT1, :])
            nc.sync.dma_start(out=stats[:], in_=ob[:])

    nc.compile()
    return nc


def _get_module():
    if "nc" not in _cached:
        _cached["nc"] = _build_module()
    return _cached["nc"]


def _shard_inputs(pred_masks, tgt_masks):
    """Relayout to per-core h-major tiles: [p, chunk, q] / [p, chunk, n+1]."""
    # h = (half*CHUNKS + c) * P + p
    pm5 = np.ascontiguousarray(
        pred_masks.reshape(B, Q, 2, CHUNKS, P).transpose(0, 2, 4, 3, 1)
    )  # (B, 2, P, CHUNKS, Q)
    tm5 = tgt_masks.reshape(B, NT, 2, CHUNKS, P).transpose(0, 2, 4, 3, 1)
    # append ones column -> (B, 2, P, CHUNKS, NT+1)
    ta = np.empty((B, 2, P, CHUNKS, NT1), np.float32)
    ta[..., :NT] = tm5
    ta[..., NT] = 1.0
    in_maps = []
    for b in range(B):
        for half in range(2):
            in_maps.append({"pmt": pm5[b, half], "taug": ta[b, half]})
    return in_maps


def _hungarian(cost):
    # Jonker-Volgenant O(n^2 m) assignment; cost (n, m), n <= m.  Same
    # algorithm/tie-breaking as the reference implementation.
    n, m = cost.shape
    INF = 1e18
    u = np.zeros(n + 1)
    v = np.zeros(m + 1)
    p = np.zeros(m + 1, dtype=np.int64)
    way = np.zeros(m + 1, dtype=np.int64)
    for i in range(1, n + 1):
        p[0] = i
        j0 = 0
        minv = np.full(m + 1, INF)
        used = np.zeros(m + 1, dtype=bool)
        while True:
            used[j0] = True
            i0 = p[j0]
            js = np.nonzero(~used[1:])[0] + 1
            cur = cost[i0 - 1, js - 1] - u[i0] - v[js]
            better = cur < minv[js]
            minv[js] = np.where(better, cur, minv[js])
            way[js[better]] = j0
            j1 = js[np.argmin(minv[js])]
            delta = minv[j1]
            u[p[used]] += delta
            v[used] -= delta
            minv[~used] -= delta
            j0 = j1
            if p[j0] == 0:
                break
        while j0:
            j1 = way[j0]
            p[j0] = p[j1]
            j0 = j1
    rows, cols = [], []
    for j in range(1, m + 1):
        if p[j]:
            rows.append(p[j] - 1)
            cols.append(j - 1)
    return np.array(rows, np.int64), np.array(cols, np.int64)


def _log_softmax(z):
    z = z - z.max(axis=-1, keepdims=True)
    return z - np.log(np.exp(z).sum(axis=-1, keepdims=True))


def kernel(pred_logits, pred_masks, tgt_labels, tgt_masks):
    pred_logits = np.asarray(pred_logits, np.float32)
    pred_masks = np.asarray(pred_masks, np.float32)
    tgt_labels = np.asarray(tgt_labels)
    tgt_masks = np.asarray(tgt_masks, np.float32)

    _ensure_path()
    from concourse.bass_utils import run_bass_kernel_spmd

    nc = _get_module()
    in_maps = _shard_inputs(pred_masks, tgt_masks)
    res = run_bass_kernel_spmd(nc, in_maps, core_ids=list(range(NCORES)))

    lab = tgt_labels.astype(np.int64)
    tsum = tgt_masks.reshape(B, NT, HW).sum(-1).astype(np.float64)

    ce_l = np.zeros(B)
    dice_l = np.zeros(B)
    bce_l = np.zeros(B)
    for b in range(B):
        s0 = res.results[2 * b]["stats"].astype(np.float64)
        s1 = res.results[2 * b + 1]["stats"].astype(np.float64)
        s = s0 + s1                                # (NT1, 3Q)
        X = s[:, 0:Q]                              # (11, 100): [n, q]
        Nm = s[:, Q:2 * Q]
        Sm = s[:, 2 * Q:3 * Q]
        Ps = X[NT] + Nm[NT]                        # (Q,) sum_h softplus(pm)
        Psig = Sm[NT]                              # (Q,) sum_h sigmoid(pm)

        cost_mask = (Ps[None, :] - X[:NT]).T / HW  # (Q, NT)
        z = pred_logits[b].astype(np.float64)
        probs = np.exp(_log_softmax(z))            # (Q, C)
        cost_class = -probs[:, lab[b]]             # (Q, NT)
        cost = np.asarray(cost_mask + cost_class, np.float64)

        t_idx, q_idx = _hungarian(cost.T)          # rows=targets, cols=queries
        r = q_idx                                  # matched queries
        c = t_idx                                  # matched targets

        logq = _log_softmax(z)                     # (Q, C)
        ce_l[b] = -logq[r, lab[b][c]].mean()
        bce_l[b] = (Ps[r] - X[c, r]).mean() / HW
        dice = (2.0 * Sm[c, r] + 1.0) / (Psig[r] + tsum[b][c] + 1.0)
        dice_l[b] = 1.0 - dice.mean()

    per_sample = ce_l + dice_l + bce_l
    return (np.float32(per_sample.mean()), np.float32(ce_l.mean()),
            np.float32(dice_l.mean()))


# revision 8
# speedup vs baseline: 2.1462x; 2.1462x over previous
"""Mask2Former loss on 8 Trainium2 NeuronCores (Bass/Tile).

Strategy
--------
All heavy compute is three batched reductions over the HW=65536 mask axis,
done as PE matmuls against the (binarized-equal-raw) target masks T plus a
ones column:

    X[n,q] = sum_h pm[q,h] * T[n,h]          (raw mask logits)
    N[n,q] = sum_h nsp[q,h] * T[n,h]         nsp = ln(1 + e^{-pm}) = softplus(-pm)
    S[n,q] = sum_h sigma[q,h] * T[n,h]       sigma = sigmoid(pm) = exp(-nsp)

with the ones column (row 10) giving the plain row sums over h.  Everything
the loss needs is a function of these (identities used):

    softplus(x) = x + softplus(-x)  =>  Ps[q] = sum_h softplus(pm) = Xs[q] + Ns[q]
    BCE_sum(q,n) = sum_h [softplus(pm) - T*pm] = Ps[q] - X[n,q]
    cost_mask = BCE_sum / HW ; bce uses the matched entries of the same matrix
    dice uses S, its ones-row, and host-side target-mask sums.

Sharding: data-parallel over B (4 samples) x 2-way split of HW -> 8 cores.
Host does: input relayout (h-major tiles so no device transposes are needed),
the tiny Hungarian assignments between device stats and final scalars, and
the O(B*Q*N) loss assembly.

The activation chain uses only Exp and Ln, which share one ACT table set
(natural_log_exp_and_others), so there is no table thrashing.  Matmuls run
in float32r (full-rate PE); all matmul operands are produced either by DMA
(raw, exact fp32 bits) or by ACT writing float32r (hardware-rounded), which
satisfies the walrus fp32r verifier.
"""

import sys
import types

import numpy as np

_TRN_REPO = "/opt/trn_rl_repo"

# Problem constants (hardcoded per contract; kernel.py must be self-contained).
B, Q, NCLS, NT, H, W = 4, 100, 3, 10, 256, 256
HW = H * W
NCORES = 8
P = 128                      # SBUF partitions / h-chunk size
CHUNKS = HW // (2 * P)       # 256 h-chunks of 128 per core (half of HW)
GROUP = 16                   # chunks per DMA/ACT group
NGROUPS = CHUNKS // GROUP    # 16
NT1 = NT + 1                 # targets + ones column

_cached = {}


def _ensure_path():
    if _TRN_REPO not in sys.path:
        sys.path.insert(0, _TRN_REPO)


def _build_module():
    """Build + compile the per-core Bass module (identical on all cores)."""
    _ensure_path()
    import concourse.bacc as bacc
    import concourse.tile as tile
    from concourse import mybir

    f32 = mybir.dt.float32
    f32r = mybir.dt.float32r
    AF = mybir.ActivationFunctionType

    # Force the act-table chooser to satisfy both Exp and Ln from the single
    # natural_log_exp_and_others set (otherwise it alternates exp_and_others /
    # natural_log and reloads tables ~2.7us on every Exp<->Ln transition).
    # Indices are preserved, so emitted act_func_set_ids still match
    # act_info.json; only the choice among candidate sets is narrowed.
    if not getattr(bacc, "_act_tables_patched", False):
        _orig_get_tables = bacc.get_activation_tables
        _keep = "natural_log_exp_and_others"
        _exp_ln = {AF.Exp, AF.Ln}

        def _patched_get_tables(arch):
            t = _orig_get_tables(arch)
            return {
                name: (fns if name == _keep else fns - _exp_ln)
                for name, fns in t.items()
            }

        bacc.get_activation_tables = _patched_get_tables
        bacc._act_tables_patched = True

    nc = bacc.Bacc("TRN2", target_bir_lowering=False, debug=False,
                   num_devices=NCORES)
    pmt = nc.dram_tensor("pmt", [P, CHUNKS, Q], f32r, kind="ExternalInput")
    taug = nc.dram_tensor("taug", [P, CHUNKS, NT1], f32r, kind="ExternalInput")
    stats = nc.dram_tensor("stats", [NT1, 3 * Q], f32, kind="ExternalOutput")

    with tile.TileContext(nc) as tc:
        with (
            tc.tile_pool(name="tpool", bufs=1) as tpool,
            tc.tile_pool(name="raws", bufs=6) as raws,
            tc.tile_pool(name="acts", bufs=3) as acts,
            tc.tile_pool(name="psum", bufs=1, space="PSUM") as psum,
            tc.tile_pool(name="outp", bufs=1) as outp,
        ):
            # One PSUM bank holds [X | N | S] side by side -> a single
            # 300-column fp32r matmul per chunk (moving dim >= 256 runs the
            # PE at full rate; 3x 100-col matmuls would run at 1/4 rate).
            ps = psum.tile([P, 3, Q], f32)

            # Groups whose sigmoid runs on DVE (reciprocal path) instead of
            # ACT (exp path): balances ACT (2 vs 3 passes/group) against the
            # otherwise-idle DVE.  sigma = 1/(1 + e2) exactly.
            DVE_SIG_GROUPS = NGROUPS * 7 // 8

            for g in range(NGROUPS):
                # st = [raw | nsp | sig] stacked in the free dim.
                st = raws.tile([P, 3, GROUP, Q], f32r)
                raw = st[:, 0, :, :]
                nsp = st[:, 1, :, :]
                sig = st[:, 2, :, :]
                nc.sync.dma_start(out=raw,
                                  in_=pmt[:, g * GROUP:(g + 1) * GROUP, :])
                if g == 0:
                    # Target masks (+ ones col), resident for the whole
                    # kernel.  Issued after the first pm group so the ACT
                    # pipeline can start filling ~4us earlier.
                    tt = tpool.tile([P, CHUNKS, NT1], f32r)
                    nc.sync.dma_start(out=tt[:], in_=taug[:])
                e2 = acts.tile([P, GROUP, Q], f32)
                # e2 = exp(-x); nsp = ln(1 + e2); sigma = exp(-nsp) = 1/(1+e2)
                nc.scalar.activation(out=e2[:], in_=raw, func=AF.Exp,
                                     scale=-1.0)
                nc.scalar.activation(out=nsp, in_=e2[:], func=AF.Ln,
                                     bias=1.0)
                if g < DVE_SIG_GROUPS:
                    t1 = acts.tile([P, GROUP, Q], f32)
                    rr = acts.tile([P, GROUP, Q], f32)
                    nc.vector.tensor_scalar_add(t1[:], e2[:], 1.0)
                    nc.vector.reciprocal_approx_fast(out=rr[:], in_=t1[:])
                    nc.vector.tensor_copy(sig, rr[:])
                else:
                    nc.scalar.activation(out=sig, in_=nsp, func=AF.Exp,
                                         scale=-1.0)
                for c in range(GROUP):
                    k = g * GROUP + c
                    nc.tensor.matmul(ps[0:NT1, :, :], tt[:, k, :],
                                     st[:, :, c, :],
                                     start=(k == 0), stop=(k == CHUNKS - 1))

            ob = outp.tile([NT1, 3 * Q], f32)
            nc.vector.tensor_copy(ob[:], ps[0:NT1, :, :])
            nc.sync.dma_start(out=stats[:], in_=ob[:])

    nc.compile()
    return nc


def _get_module():
    if "nc" not in _cached:
        _cached["nc"] = _build_module()
    return _cached["nc"]


def _shard_inputs(pred_masks, tgt_masks):
    """Relayout to per-core h-major tiles: [p, chunk, q] / [p, chunk, n+1]."""
    # h = (half*CHUNKS + c) * P + p
    pm5 = np.ascontiguousarray(
        pred_masks.reshape(B, Q, 2, CHUNKS, P).transpose(0, 2, 4, 3, 1)
    )  # (B, 2, P, CHUNKS, Q)
    tm5 = tgt_masks.reshape(B, NT, 2, CHUNKS, P).transpose(0, 2, 4, 3, 1)
    # append ones column -> (B, 2, P, CHUNKS, NT+1)
    ta = np.empty((B, 2, P, CHUNKS, NT1), np.float32)
    ta[..., :NT] = tm5
    ta[..., NT] = 1.0
    in_maps = []
    for b in range(B):
        for half in range(2):
            in_maps.append({"pmt": pm5[b, half], "taug": ta[b, half]})
    return in_maps


def _hungarian(cost):
    # Jonker-Volgenant O(n^2 m) assignment; cost (n, m), n <= m.  Same
    # algorithm/tie-breaking as the reference implementation.
    n, m = cost.shape
    INF = 1e18
    u = np.zeros(n + 1)
    v = np.zeros(m + 1)
    p = np.zeros(m + 1, dtype=np.int64)
    way = np.zeros(m + 1, dtype=np.int64)
    for i in range(1, n + 1):
        p[0] = i
        j0 = 0
        minv = np.full(m + 1, INF)
        used = np.zeros(m + 1, dtype=bool)
        while True:
            used[j0] = True
            i0 = p[j0]
            js = np.nonzero(~used[1:])[0] + 1
            cur = cost[i0 - 1, js - 1] - u[i0] - v[js]
            better = cur < minv[js]
            minv[js] = np.where(better, cur, minv[js])
            way[js[better]] = j0
            j1 = js[np.argmin(minv[js])]
            delta = minv[j1]
            u[p[used]] += delta
            v[used] -= delta
            minv[~used] -= delta
            j0 = j1
            if p[j0] == 0:
                break
        while j0:
            j1 = way[j0]
            p[j0] = p[j1]
            j0 = j1
    rows, cols = [], []
    for j in range(1, m + 1):
        if p[j]:
            rows.append(p[j] - 1)
            cols.append(j - 1)
    return np.array(rows, np.int64), np.array(cols, np.int64)


def _log_softmax(z):
    z = z - z.max(axis=-1, keepdims=True)
    return z - np.log(np.exp(z).sum(axis=-1, keepdims=True))


def kernel(pred_logits, pred_masks, tgt_labels, tgt_masks):
    pred_logits = np.asarray(pred_logits, np.float32)
    pred_masks = np.asarray(pred_masks, np.float32)
    tgt_labels = np.asarray(tgt_labels)
    tgt_masks = np.asarray(tgt_masks, np.float32)

    _ensure_path()
    from concourse.bass_utils import run_bass_kernel_spmd

    nc = _get_module()
    in_maps = _shard_inputs(pred_masks, tgt_masks)
    res = run_bass_kernel_spmd(nc, in_maps, core_ids=list(range(NCORES)))

    lab = tgt_labels.astype(np.int64)
    tsum = tgt_masks.reshape(B, NT, HW).sum(-1).astype(np.float64)

    ce_l = np.zeros(B)
    dice_l = np.zeros(B)
    bce_l = np.zeros(B)
    for b in range(B):
        s0 = res.results[2 * b]["stats"].astype(np.float64)
        s1 = res.results[2 * b + 1]["stats"].astype(np.float64)
        s = s0 + s1                                # (NT1, 3Q)
        X = s[:, 0:Q]                              # (11, 100): [n, q]
        Nm = s[:, Q:2 * Q]
        Sm = s[:, 2 * Q:3 * Q]
        Ps = X[NT] + Nm[NT]                        # (Q,) sum_h softplus(pm)
        Psig = Sm[NT]                              # (Q,) sum_h sigmoid(pm)

        cost_mask = (Ps[None, :] - X[:NT]).T / HW  # (Q, NT)
        z = pred_logits[b].astype(np.float64)
        probs = np.exp(_log_softmax(z))            # (Q, C)
        cost_class = -probs[:, lab[b]]             # (Q, NT)
        cost = np.asarray(cost_mask + cost_class, np.float64)

        t_idx, q_idx = _hungarian(cost.T)          # rows=targets, cols=queries
        r = q_idx                                  # matched queries
        c = t_idx                                  # matched targets

        logq = _log_softmax(z)                     # (Q, C)
        ce_l[b] = -logq[r, lab[b][c]].mean()
        bce_l[b] = (Ps[r] - X[c, r]).mean() / HW
        dice = (2.0 * Sm[c, r] + 1.0) / (Psig[r] + tsum[b][c] + 1.0)
        dice_l[b] = 1.0 - dice.mean()

    per_sample = ce_l + dice_l + bce_l
    return (np.float32(per_sample.mean()), np.float32(ce_l.mean()),
            np.float32(dice_l.mean()))
